# revision 1
# baseline (speedup 1.0000x reference)
"""Trainium2 Bass kernel for nn_Dihedral (gnn_message_passing, 8 NeuronCores).

kernel(**inputs) -> [256] f32 per-batch dihedral energies.

Design: mapping columns are consecutive-atom windows (b..b+3), so every
per-dihedral quantity except the batch label is a function of the window
start.  Host prep builds a 256 B per-window record table
    [pos[w..w+3] (12 f32), -k*cos(th) (3), -k*sin(th) (3), sum_k (1), pad]
(256 B = the dma_gather granularity), shards windows by atom range across the
8 cores (4 int16-addressable subtables each, per the shard-the-dihedral-dim
hint, with the small theta/k tables folded into the records), and routes each
dihedral as an int16 record index to the core owning its window, ordered by
batch with bins padded to 32-element blocks.

Device per core: tiles of 128*F dihedrals; dma_gather (4 SWDGE queues,
<=8192 idxs/call) pulls records into SBUF; DVE/ACT compute the torsion
(x = n1.n2, y = -(a.n2)*|b|; cos = x/r, sin = y/r, Chebyshev for d=2,3 --
no atan/cos tables needed); PE reduces each 32-element block of V via a
[128,4] indicator matmul.  Host: bincount block sums into the 256 bins and
sum the 8 per-core partials (the all-reduce of the sum-sharded output).
"""

import sys
import numpy as np

if "/opt/trn_rl_repo" not in sys.path:
    sys.path.insert(0, "/opt/trn_rl_repo")

import concourse.bass as bass
import concourse.bacc as bacc
import concourse.mybir as mybir
import concourse.tile as tile
from concourse.library_config import mlp
from concourse.tile_rust import add_dep_helper

P = 128
ELEM = 64            # f32 slots per record = 256B
NCORES = 8
QUANT = 32           # bin padding quantum == PE group size
NGRP = P // QUANT    # 4 partial sums per column


# --------------------------------------------------------------------------
# host-side prep
# --------------------------------------------------------------------------

def build_record_table(pos, atom_types, thetas, ks):
    """[NW, 19] f32 window records (not yet sharded/padded)."""
    NW = pos.shape[0] - 3
    t3 = thetas.reshape(3, -1).astype(np.float64)
    k3 = ks.reshape(3, -1).astype(np.float64)
    A = (-(k3 * np.cos(t3))).astype(np.float32)      # [3, 390625]
    B = (-(k3 * np.sin(t3))).astype(np.float32)
    C = k3.sum(axis=0).astype(np.float32)            # [390625]
    ty = np.asarray(atom_types).astype(np.int64)
    T4 = ((ty[:NW] * 25 + ty[1:NW + 1]) * 25 + ty[2:NW + 2]) * 25 + ty[3:NW + 3]
    rec = np.empty((NW, 19), dtype=np.float32)
    rec[:, 0:3] = pos[0:NW]
    rec[:, 3:6] = pos[1:NW + 1]
    rec[:, 6:9] = pos[2:NW + 2]
    rec[:, 9:12] = pos[3:NW + 3]
    rec[:, 12] = A[0, T4]; rec[:, 13] = A[1, T4]; rec[:, 14] = A[2, T4]
    rec[:, 15] = B[0, T4]; rec[:, 16] = B[1, T4]; rec[:, 17] = B[2, T4]
    rec[:, 18] = C[T4]
    return rec


def plan_layout(base, batch, n_win, F, gcall=64):
    """Shard dihedrals by window range; per (core, subtable) order by batch,
    pad bins to QUANT and groups to P, sizes common across cores.

    Returns dict with per-core index arrays, call plan, and block labels."""
    NSUB = 4
    SUBT = (n_win + NCORES * NSUB - 1) // (NCORES * NSUB)   # rows per subtable
    assert SUBT <= 32000, SUBT
    DUMMY = SUBT                                            # dummy record slot
    SUBT_ALLOC = ((SUBT + 1 + 63) // 64) * 64               # dram rows per sub

    sub_g = base // SUBT                 # global subtable 0..31
    local = (base - sub_g * SUBT).astype(np.int32)
    core_of = sub_g // NSUB
    sub_l = sub_g % NSUB

    # per (core, sub): element lists (already batch-sorted since input order is)
    per = {}
    for c in range(NCORES):
        for s in range(NSUB):
            sel = np.nonzero((core_of == c) & (sub_l == s))[0]
            per[(c, s)] = sel

    # per (core, sub): pad each batch-bin to a QUANT multiple
    padded_idx = {}      # int16 record index stream (DUMMY for padding)
    padded_lab = {}      # per-QUANT-block batch label
    for (c, s), sel in per.items():
        lab = batch[sel]
        cnt = np.bincount(lab, minlength=256)
        pcnt = ((cnt + QUANT - 1) // QUANT) * QUANT
        total = int(pcnt.sum())
        idx_out = np.full(total, DUMMY, dtype=np.int16)
        ends = np.cumsum(pcnt)
        starts = ends - pcnt
        # positions of real elements: starts[lab] + rank within bin
        within = np.arange(len(sel)) - np.repeat(np.cumsum(cnt) - cnt, cnt)
        pos_out = starts[lab] + within
        idx_out[pos_out] = local[sel].astype(np.int16)
        blk_lab = np.repeat(np.arange(256), pcnt // QUANT)
        padded_idx[(c, s)] = idx_out
        padded_lab[(c, s)] = blk_lab

    # common per-sub column counts across cores (pad with full-dummy cols)
    ncols_sub = []
    for s in range(4):
        m = max(len(padded_idx[(c, s)]) for c in range(NCORES))
        ncols_sub.append((m + P - 1) // P)

    # gather-call plan: (sub, cols<=gcall) chunks; compute groups pack
    # consecutive calls up to F columns
    calls = []
    for s in range(4):
        rem = ncols_sub[s]
        while rem > 0:
            f = min(gcall, rem)
            calls.append((s, f))
            rem -= f
    ncols_tot = sum(f for _, f in calls)
    groups = []
    cur = []
    cur_cols = 0
    for (s, f) in calls:
        if cur and cur_cols + f > F:
            groups.append(cur)
            cur, cur_cols = [], 0
        cur.append((s, f))
        cur_cols += f
    if cur:
        groups.append(cur)

    # per-core wrapped idx stream + block labels
    idx_dram = np.zeros((NCORES, P, 8 * ncols_tot), dtype=np.int16)
    blk_labels = np.full((NCORES, ncols_tot * NGRP), -1, dtype=np.int32)
    for c in range(NCORES):
        col0 = 0
        for s in range(4):
            arr = padded_idx[(c, s)]
            full = np.full(ncols_sub[s] * P, DUMMY, dtype=np.int16)
            full[:len(arr)] = arr
            lab = padded_lab[(c, s)]
            labfull = np.full(ncols_sub[s] * NGRP, -1, dtype=np.int32)
            labfull[:len(lab)] = lab
            blk_labels[c, col0 * NGRP:(col0 + ncols_sub[s]) * NGRP] = labfull
            # wrapped layout per call
            cc = col0
            off = 0
            for (ss, f) in calls:
                if ss != s:
                    continue
                chunk = full[off:off + f * P]
                if len(chunk) < f * P:
                    break
                wrapped = chunk.reshape(8 * f, 16).T          # [16, 8f]
                idx_dram[c, :, cc * 8:(cc + f) * 8] = np.tile(wrapped, (8, 1))
                off += f * P
                cc += f
            col0 += ncols_sub[s]
    return dict(SUBT=SUBT, SUBT_ALLOC=SUBT_ALLOC, DUMMY=DUMMY, calls=calls,
                groups=groups, ncols_tot=ncols_tot, idx_dram=idx_dram,
                blk_labels=blk_labels, ncols_sub=ncols_sub)


def build_core_tables(rec, plan):
    """[NCORES, 4*SUBT_ALLOC, ELEM] f32 sharded record tables."""
    SUBT, SUBT_ALLOC, DUMMY = plan["SUBT"], plan["SUBT_ALLOC"], plan["DUMMY"]
    NW = rec.shape[0]
    tables = np.zeros((NCORES, 4 * SUBT_ALLOC, ELEM), dtype=np.float32)
    # dummy record: valid geometry (window 0), zero coefficients
    dummy_rec = np.zeros(ELEM, dtype=np.float32)
    dummy_rec[:12] = rec[0, :12]
    for c in range(NCORES):
        for s in range(4):
            g = c * 4 + s
            lo = g * SUBT
            hi = min(lo + SUBT, NW)
            if hi > lo:
                tables[c, s * SUBT_ALLOC: s * SUBT_ALLOC + (hi - lo), :19] = rec[lo:hi]
            tables[c, s * SUBT_ALLOC + DUMMY] = dummy_rec
    return tables


# --------------------------------------------------------------------------
# device program
# --------------------------------------------------------------------------

def build_program(plan, repeat=1, mode="full", nqueues=1):
    SUBT_ALLOC = plan["SUBT_ALLOC"]
    groups = plan["groups"]
    ncols_tot = plan["ncols_tot"]
    f32 = mybir.dt.float32
    Alu = mybir.AluOpType
    Act = mybir.ActivationFunctionType

    nc = bacc.Bacc("TRN2", target_bir_lowering=False, debug=False,
                   num_swdge_queues=nqueues)
    tbl = nc.dram_tensor("tbl", [4 * SUBT_ALLOC, ELEM], f32, kind="ExternalInput").ap()
    idxs = nc.dram_tensor("idxs", [P, 8 * ncols_tot], mybir.dt.int16,
                          kind="ExternalInput").ap()
    out = nc.dram_tensor("out", [NGRP, ncols_tot], f32, kind="ExternalOutput").ap()

    with tile.TileContext(nc) as tc:
        with (
            tc.tile_pool(name="gat", bufs=3) as gat_pool,
            tc.tile_pool(name="tmp", bufs=1) as tmp_pool,
            tc.tile_pool(name="cst", bufs=1) as cst_pool,
            tc.tile_pool(name="ps", bufs=2, space="PSUM") as ps_pool,
        ):
            lib_inst = nc.gpsimd.load_library(mlp)

            grp = cst_pool.tile([P, NGRP], f32)
            nc.gpsimd.memset(grp[:], 0.0)
            for g in range(NGRP):
                nc.gpsimd.memset(grp[g * QUANT:(g + 1) * QUANT, g:g + 1], 1.0)

            bs = cst_pool.tile([NGRP, ncols_tot], f32)

            first_gather = [None]
            call_no = [0]

            def do_group(group, col0):
                F_j = sum(f for _, f in group)
                g = gat_pool.tile([P, F_j * ELEM], f32, tag="g")
                off = 0
                for (sub, f) in group:
                    if mode == "compute":
                        off += f
                        continue
                    N = P * f
                    it = tmp_pool.tile([P, 8 * f], mybir.dt.int16, tag="idx",
                                       name="idx", bufs=8)
                    nc.sync.dma_start(
                        out=it[:],
                        in_=idxs[:, (col0 + off) * 8:(col0 + off + f) * 8])
                    gi = nc.gpsimd.dma_gather(
                        g[:, off * ELEM:(off + f) * ELEM].rearrange(
                            "p (f e) -> p f e", e=ELEM),
                        tbl[sub * SUBT_ALLOC:(sub + 1) * SUBT_ALLOC, :],
                        it[:], N, N, ELEM, single_packet=False,
                        queue_num=call_no[0] % nqueues,
                    )
                    call_no[0] += 1
                    if first_gather[0] is None:
                        first_gather[0] = gi
                        add_dep_helper(lib_inst.ins, gi.ins, sync=False,
                                       reason="library before gather")
                    off += f
                if mode == "compute":
                    nc.vector.memset(g[:], 1.0)

                ge = g[:].rearrange("p (f e) -> p e f", e=ELEM)

                def fld(k):
                    return ge[:, k, :]

                if mode == "gather":
                    pt0 = ps_pool.tile([NGRP, F_j], f32, tag="ps", name="ps")
                    nc.tensor.matmul(out=pt0[:], lhsT=grp[:], rhs=fld(0),
                                     start=True, stop=True)
                    nc.scalar.activation(bs[:, col0:col0 + F_j], pt0[:], Act.Copy)
                    return

                def T(tag):
                    return tmp_pool.tile([P, F_j], f32, tag=tag, name=tag)

                def tt(o, i0, i1, op):
                    nc.vector.tensor_tensor(out=o, in0=i0, in1=i1, op=op)

                # dr vectors (strided reads from the gathered records)
                abc = []
                for j, (hi, lo) in enumerate([(3, 0), (6, 3), (9, 6)]):
                    for k in range(3):
                        t = T(f"d{j}{k}")
                        tt(t[:], fld(hi + k), fld(lo + k), Alu.subtract)
                        abc.append(t)
                ax, ay, az, bx, by, bz, cx, cy, cz = abc

                def cross(ux, uy, uz, vx, vy, vz, tag):
                    o = []
                    for k, (m1, m2, m3, m4) in enumerate([
                            (uy, vz, uz, vy), (uz, vx, ux, vz), (ux, vy, uy, vx)]):
                        t1 = T(f"{tag}t{k}")
                        t2 = T(f"{tag}u{k}")
                        tt(t1[:], m1[:], m2[:], Alu.mult)
                        tt(t2[:], m3[:], m4[:], Alu.mult)
                        tt(t1[:], t1[:], t2[:], Alu.subtract)
                        o.append(t1)
                    return o

                n1 = cross(ax, ay, az, bx, by, bz, "n1")
                n2 = cross(bx, by, bz, cx, cy, cz, "n2")

                def dot(u, v, tag):
                    acc = T(tag)
                    t = T(tag + "t")
                    tt(acc[:], u[0][:], v[0][:], Alu.mult)
                    tt(t[:], u[1][:], v[1][:], Alu.mult)
                    tt(acc[:], acc[:], t[:], Alu.add)
                    tt(t[:], u[2][:], v[2][:], Alu.mult)
                    tt(acc[:], acc[:], t[:], Alu.add)
                    return acc

                x = dot(n1, n2, "x")
                D = dot([ax, ay, az], n2, "D")
                w = dot([bx, by, bz], [bx, by, bz], "w")
                L = T("L"); nc.scalar.sqrt(L[:], w[:])
                p = T("p"); tt(p[:], D[:], L[:], Alu.mult)
                qa = T("qa"); nc.scalar.square(qa[:], x[:])
                qb = T("qb"); nc.scalar.square(qb[:], p[:])
                tt(qa[:], qa[:], qb[:], Alu.add)
                r = T("r"); nc.scalar.sqrt(r[:], qa[:])
                inv = T("inv"); nc.vector.reciprocal_approx_fast(inv[:], r[:])
                c1 = T("c1"); tt(c1[:], x[:], inv[:], Alu.mult)
                s1 = T("s1")
                nc.vector.scalar_tensor_tensor(
                    out=s1[:], in0=p[:], scalar=-1.0, in1=inv[:],
                    op0=Alu.mult, op1=Alu.mult)
                cc = T("cc"); nc.scalar.square(cc[:], c1[:])
                c2 = T("c2")
                nc.scalar.activation(c2[:], cc[:], Act.Copy, bias=-1.0, scale=2.0)
                s2 = T("s2")
                nc.vector.scalar_tensor_tensor(
                    out=s2[:], in0=s1[:], scalar=2.0, in1=c1[:],
                    op0=Alu.mult, op1=Alu.mult)
                r3 = T("r3")
                nc.scalar.activation(r3[:], cc[:], Act.Copy, bias=-3.0, scale=4.0)
                c3 = T("c3"); tt(c3[:], r3[:], c1[:], Alu.mult)
                r4 = T("r4")
                nc.scalar.activation(r4[:], cc[:], Act.Copy, bias=-1.0, scale=4.0)
                s3 = T("s3"); tt(s3[:], r4[:], s1[:], Alu.mult)

                V = T("V"); t = T("Vt")
                tt(V[:], fld(12), c1[:], Alu.mult)          # A1*c1
                tt(V[:], V[:], fld(18), Alu.add)            # + C
                for fk, cs in [(15, s1), (13, c2), (16, s2), (14, c3), (17, s3)]:
                    tt(t[:], fld(fk), cs[:], Alu.mult)
                    tt(V[:], V[:], t[:], Alu.add)

                pt = ps_pool.tile([NGRP, F_j], f32, tag="ps")
                nc.tensor.matmul(out=pt[:], lhsT=grp[:], rhs=V[:],
                                 start=True, stop=True)
                nc.scalar.activation(bs[:, col0:col0 + F_j], pt[:], Act.Copy)

            def body():
                col0 = 0
                for grp_calls in groups:
                    do_group(grp_calls, col0)
                    col0 += sum(f for _, f in grp_calls)

            if repeat > 1:
                with tc.For_i(0, repeat, 1):
                    body()
            else:
                body()

            nc.sync.dma_start(out=out[:], in_=bs[:])
    nc.compile()
    return nc


# --------------------------------------------------------------------------
# end to end
# --------------------------------------------------------------------------

def prepare(inputs, F=192, gcall=64):
    pos = np.asarray(inputs["pos"], dtype=np.float32)
    ty = np.asarray(inputs["atom_types"])
    mapping = np.asarray(inputs["mapping"])
    batch = np.asarray(inputs["mapping_batch"]).astype(np.int64)
    base = np.asarray(mapping[0]).astype(np.int64)
    assert all(np.array_equal(np.asarray(mapping[j]), base + j) for j in range(1, 4)), \
        "mapping not consecutive; fast path invalid"
    n_win = pos.shape[0] - 3
    rec = build_record_table(pos, ty, np.asarray(inputs["thetas"]),
                             np.asarray(inputs["ks"]))
    plan = plan_layout(base, batch, n_win, F, gcall=gcall)
    tables = build_core_tables(rec, plan)
    return plan, tables


def finish(plan, outs, n_batch=256):
    """outs: list per core of [NGRP, ncols_tot] block sums -> [256] energy."""
    energy = np.zeros(n_batch, dtype=np.float64)
    for c in range(NCORES):
        bsums = np.asarray(outs[c])          # [NGRP, ncols]
        lab = plan["blk_labels"][c]          # [ncols*NGRP], -1 = padding
        vals = bsums.T.ravel()               # block (col,grp) order
        m = lab >= 0
        energy += np.bincount(lab[m], weights=vals[m].astype(np.float64),
                              minlength=n_batch)
    return energy.astype(np.float32)


def _kernel_numpy_fallback(pos, atom_types, mapping, mapping_batch, thetas, ks):
    # Correctness safety net for non-consecutive mappings (never expected).
    p0, p1 = pos[mapping[0]], pos[mapping[1]]
    p2, p3 = pos[mapping[2]], pos[mapping[3]]
    dr1, dr2, dr3 = p1 - p0, p2 - p1, p3 - p2
    n1 = np.cross(dr1, dr2); n2 = np.cross(dr2, dr3)
    m1 = np.cross(n1, dr2 / np.linalg.norm(dr2, axis=-1, keepdims=True))
    x = np.sum(n1 * n2, -1); y = np.sum(m1 * n2, -1)
    theta = np.arctan2(y, x)
    t0, t1, t2, t3 = (atom_types[mapping[j]] for j in range(4))
    th = thetas[:, t0, t1, t2, t3]; kk = ks[:, t0, t1, t2, t3]
    degs = np.arange(1, 4)[:, None]
    V = np.sum(kk * (1.0 - np.cos(degs * theta[None, :] - th)), axis=0)
    return np.bincount(mapping_batch, weights=V.astype(np.float64),
                       minlength=256).astype(np.float32)


def kernel(pos, atom_types, mapping, mapping_batch, thetas, ks):
    from concourse.bass_utils import run_bass_kernel_spmd
    pos = np.asarray(pos, dtype=np.float32)
    atom_types = np.asarray(atom_types)
    mapping = np.asarray(mapping)
    mapping_batch = np.asarray(mapping_batch)
    thetas = np.asarray(thetas, dtype=np.float32)
    ks = np.asarray(ks, dtype=np.float32)

    base = np.asarray(mapping[0]).astype(np.int64)
    if not all(np.array_equal(np.asarray(mapping[j]), base + j)
               for j in range(1, 4)):
        print("kernel.py: non-consecutive mapping; numpy fallback",
              file=sys.stderr)
        return _kernel_numpy_fallback(pos, atom_types, mapping, mapping_batch,
                                      thetas, ks)

    batch = mapping_batch.astype(np.int64)
    n_win = pos.shape[0] - 3
    rec = build_record_table(pos, atom_types, thetas, ks)
    plan = plan_layout(base, batch, n_win, F=176, gcall=16)
    tables = build_core_tables(rec, plan)

    nc = build_program(plan, repeat=1, mode="full", nqueues=4)
    in_maps = [{"tbl": tables[c], "idxs": plan["idx_dram"][c]}
               for c in range(NCORES)]
    res = run_bass_kernel_spmd(nc, in_maps, list(range(NCORES)))
    outs = [res.results[c]["out"] for c in range(NCORES)]
    return finish(plan, outs).astype(np.float32)

